# revision 19
# baseline (speedup 1.0000x reference)
"""ANFIS kernel for 8 TRN2 NeuronCores — pure batch data-parallel.

Math: out[b,o] = prod_f(x[b,f]) * w[b,o]^32 where
  w = sum_r(p_r * m_r) / sum_r(m_r),  m_r = exp(-((y-c_r)/s_r)^2),
  y = MLP(x).  exp(-z^2) is computed on the ScalarEngine as
  Derivative_Erf(scale*y + bias) (= 2/sqrt(pi) * exp(-z^2); the constant
  cancels in the normalization).  D = sum m and N = sum p*m are reduced
  over rules with fp16 TensorE matmuls (identity / diag(p) stationary),
  accumulating in f32 PSUM.

Schedule: ACT (derf) is the bottleneck engine (32 x ~1.24us passes); the
kernel keeps ACT saturated: inputs DMA'd in dependency order (small-row
tensors separated so they don't starve the fat ones), junk matmuls keep
the PE clock-gate (HAM) warm through the MLP ladder, the N-reduction
lags D by 3 rules in o-tile 0 (late diag DMA), and the last o-tile uses
per-chunk PSUM/SBUF tiles with its trailing rules' derfs split c1-block-
first so the c1 normalization/power chain overlaps the c0 derf stream.
"""
import sys

if "/opt/trn_rl_repo" not in sys.path:
    sys.path.insert(0, "/opt/trn_rl_repo")

import numpy as np
import ml_dtypes
ml_bf16 = ml_dtypes.bfloat16

import concourse.bacc as bacc
import concourse.mybir as mybir
from concourse.bass_utils import run_bass_kernel_spmd
from concourse.tile import TileContext
from concourse.mybir import AluOpType as Op

B, IN_DIM, OUT_DIM, N_RULES, H = 8192, 32, 256, 16, 256
N_CORES = 8
BL = B // N_CORES          # 1024 batch rows per core
P = 128                    # partitions
NOT = OUT_DIM // P         # 2 o-tiles
NJ = H // P                # 2 hidden j-tiles
FD = 512                   # matmul free-dim chunk (one PSUM bank)
NSPL = 4                   # trailing rules of the last o-tile chunk-split
NLAG = 3                   # N-matmul lag behind D in o-tile 0 (dg0 DMA is late)
F32 = mybir.dt.float32
F16 = mybir.dt.float16
BF16 = mybir.dt.bfloat16

# small f32 constant columns: b1t | b2t | scl | bia
C_B1 = 0
C_B2 = C_B1 + NJ
C_SCL = C_B2 + NJ
C_BIA = C_SCL + NOT * N_RULES
C_CS = C_BIA + NOT * N_RULES

_nc_cache = None


def _build():
    global _nc_cache
    if _nc_cache is not None:
        return _nc_cache
    nc = bacc.Bacc(None, target_bir_lowering=False, debug=False, num_devices=N_CORES)

    w1s_d = nc.declare_dram_parameter("w1s", [3 * IN_DIM, H], BF16, isOutput=False)
    xw_d = nc.declare_dram_parameter("xw", [3 * IN_DIM, BL], BF16, isOutput=False)
    cs_d = nc.declare_dram_parameter("cs", [P, C_CS], F32, isOutput=False)
    w2_d = nc.declare_dram_parameter("w2", [P, NJ * NJ * P], F16, isOutput=False)
    w3_d = nc.declare_dram_parameter("w3", [P, NJ * NOT * P], F16, isOutput=False)
    ey_d = nc.declare_dram_parameter("ey", [P, P], F16, isOutput=False)
    dg0_d = nc.declare_dram_parameter("dg0", [P, N_RULES * P], F16, isOutput=False)
    xbp_d = nc.declare_dram_parameter("xbp", [P, (BL // P) * IN_DIM], F32, isOutput=False)
    dg1_d = nc.declare_dram_parameter("dg1", [P, N_RULES * P], F16, isOutput=False)
    out_d = nc.declare_dram_parameter("out", [OUT_DIM, BL], BF16, isOutput=True)

    DERF = mybir.ActivationFunctionType.Derivative_Erf
    SQ = mybir.ActivationFunctionType.Square
    RELU = mybir.ActivationFunctionType.Relu
    NCH = BL // FD  # chunks
    c0 = slice(0, FD)
    c1 = slice(FD, BL)

    with TileContext(nc) as tc:
        with tc.sbuf_pool(name="sb", bufs=1) as sb:
            # ---- loads, in dependency order (issue order = priority) ----
            w1s = sb.tile([3 * IN_DIM, H], BF16)
            nc.sync.dma_start(out=w1s[:], in_=w1s_d[:])
            cs = sb.tile([P, C_CS], F32)
            nc.sync.dma_start(out=cs[:], in_=cs_d[:])
            xw = sb.tile([3 * IN_DIM, BL], BF16)
            nc.sync.dma_start(out=xw[:], in_=xw_d[:])
            w2 = sb.tile([P, NJ * NJ * P], F16)
            nc.sync.dma_start(out=w2[:], in_=w2_d[:])
            w3 = sb.tile([P, NJ * NOT * P], F16)
            nc.sync.dma_start(out=w3[:], in_=w3_d[:])
            eye16 = sb.tile([P, P], F16)
            nc.sync.dma_start(out=eye16[:], in_=ey_d[:])
            dg0 = sb.tile([P, N_RULES * P], F16)
            nc.sync.dma_start(out=dg0[:], in_=dg0_d[:])
            xbp = sb.tile([P, (BL // P) * IN_DIM], F32)
            nc.sync.dma_start(out=xbp[:], in_=xbp_d[:])
            dg1 = sb.tile([P, N_RULES * P], F16)
            nc.sync.dma_start(out=dg1[:], in_=dg1_d[:])

            # ---- junk tile for HAM (PE clock-gate) keep-warm matmuls; the
            # dummy derf forces the erf_derivative table set (which covers
            # Relu/Square too) to load during the preamble ----
            junk16 = sb.tile([P, P + FD], F16)
            nc.vector.memset(junk16[:], 0.0)
            nc.scalar.activation(junk16[:, 0:8], junk16[:, 0:8], DERF)

            b1t = cs[:, C_B1:C_B1 + NJ]
            b2t = cs[:, C_B2:C_B2 + NJ]
            scl = cs[:, C_SCL:C_SCL + NOT * N_RULES]
            bia = cs[:, C_BIA:C_BIA + NOT * N_RULES]

            def dgblk(ot, r):
                src = dg0 if ot == 0 else dg1
                return src[:, r * P:(r + 1) * P]

            def W2blk(k, j):
                return w2[:, (k * NJ + j) * P:(k * NJ + j + 1) * P]

            def W3blk(k, j):
                return w3[:, (k * NOT + j) * P:(k * NOT + j + 1) * P]

            def relu_bias(dst, src_psum, bias_col, j, cc):
                # chunked; one engine per j-tile so the two run in parallel
                if j % 2 == 0:
                    nc.vector.tensor_scalar(dst[:, cc], src_psum[:, cc], bias_col, 0.0,
                                            Op.add, Op.max)
                else:
                    nc.scalar.activation(dst[:, cc], src_psum[:, cc], RELU,
                                         bias=bias_col, scale=1.0)

            # ---- MLP (chunk-pipelined; junk MMs keep HAM warm in the gaps) ----
            hT = []
            h2T = []
            with tc.psum_pool(name="ps_warm", bufs=1) as ps_warm:
                wt = ps_warm.tile([P, FD], F32, tag="warm")

                def keep_warm(n):
                    for _ in range(n):
                        nc.tensor.matmul(wt[:], junk16[:, :P], junk16[:, P:],
                                         start=True, stop=True)

                keep_warm(3)
                with tc.psum_pool(name="ps_mlp", bufs=2) as ps_mlp:
                    l1T = []
                    for j in range(NJ):
                        l1T.append(ps_mlp.tile([P, BL], F32, tag="mlp", name=f"l1T{j}"))
                        hT.append(sb.tile([P, BL], F16, name=f"hT{j}"))
                    for c in range(NCH):
                        cc = slice(c * FD, (c + 1) * FD)
                        for j in range(NJ):
                            nc.tensor.matmul(
                                l1T[j][:, cc],
                                w1s[:, j * P:(j + 1) * P],
                                xw[:, cc],
                                start=True, stop=True,
                            )
                        keep_warm(2)
                        for j in range(NJ):
                            relu_bias(hT[j], l1T[j], b1t[:, j:j + 1], j, cc)
                    l2T = []
                    for j in range(NJ):
                        l2T.append(ps_mlp.tile([P, BL], F32, tag="mlp", name=f"l2T{j}"))
                        h2T.append(sb.tile([P, BL], F16, name=f"h2T{j}"))
                    for c in range(NCH):
                        cc = slice(c * FD, (c + 1) * FD)
                        for k in range(NJ):
                            for j in range(NJ):
                                nc.tensor.matmul(
                                    l2T[j][:, cc],
                                    W2blk(k, j),
                                    hT[k][:, cc],
                                    start=(k == 0), stop=(k == NJ - 1),
                                )
                        keep_warm(2)
                        for j in range(NJ):
                            relu_bias(h2T[j], l2T[j], b2t[:, j:j + 1], j, cc)
                    keep_warm(2)

            # ---- P[b] = prod_f x[b,f] (issued after relus in DVE order so it
            # can wait on the late xbp DMA without blocking the MLP) ----
            P_all = sb.tile([P, BL // P], F32)
            nc.vector.tensor_reduce(
                P_all[:],
                xbp.rearrange("p (t f) -> p t f", f=IN_DIM),
                mybir.AxisListType.X, Op.mult,
            )
            P_row = sb.tile([1, BL], F32)
            for t in range(BL // P):
                nc.sync.dma_start(out=P_row[0:1, t * P:(t + 1) * P], in_=P_all[:, t:t + 1])
            P_rep = sb.tile([P, BL], F32)
            nc.gpsimd.partition_broadcast(P_rep[:], P_row[0:1, :])

            # ---- L3 into PSUM (y stays there: ACT PSUM reads are fast) ----
            with tc.psum_pool(name="ps_y", bufs=2) as ps_y:
                yT = []
                for j in range(NOT):
                    l3 = ps_y.tile([P, BL], F32, tag="yt")
                    for c in range(NCH):
                        cc = slice(c * FD, (c + 1) * FD)
                        for k in range(NJ):
                            nc.tensor.matmul(
                                l3[:, cc],
                                W3blk(k, j),
                                h2T[k][:, cc],
                                start=(k == 0), stop=(k == NJ - 1),
                            )
                    yT.append(l3)

                # ---- memberships + D/N + w per o-tile ----
                # last o-tile uses per-chunk tiles so the c1 chain has no
                # false deps on c0 writes (tile-granular dep tracking)
                with tc.psum_pool(name="ps_dn", bufs=1) as ps_dn:
                    for ot in range(NOT):
                        lastot = (ot == NOT - 1)
                        Dt = {c0: ps_dn.tile([P, FD], F32, tag="Dc0", name=f"D{ot}c0"),
                              c1: ps_dn.tile([P, FD], F32, tag="Dc1", name=f"D{ot}c1")}
                        Nt = {c0: ps_dn.tile([P, FD], F32, tag="Nc0", name=f"N{ot}c0"),
                              c1: ps_dn.tile([P, FD], F32, tag="Nc1", name=f"N{ot}c1")}

                        mtiles = {}

                        def derf_m(r, cc):
                            idx = ot * N_RULES + r
                            nc.scalar.activation(
                                mtiles[r][:, cc], yT[ot][:, cc], DERF,
                                bias=bia[:, idx:idx + 1], scale=scl[:, idx:idx + 1])

                        def mm_d(r, cc):
                            nc.tensor.matmul(
                                Dt[cc][:, :], eye16[:], mtiles[r][:, cc],
                                start=(r == 0), stop=(r == N_RULES - 1))

                        def mm_n(r, cc):
                            nc.tensor.matmul(
                                Nt[cc][:, :], dgblk(ot, r), mtiles[r][:, cc],
                                start=(r == 0), stop=(r == N_RULES - 1))

                        nfull = N_RULES - (NSPL if lastot else 0)
                        lag = NLAG if ot == 0 else 0
                        nent = 3 if ot == 0 else 0
                        # o-tile 0 entry: first rules' c0 derfs run as a block
                        # as soon as yT0's c0 half exists (one ladder stage
                        # before c1) — fills otherwise-idle ACT time
                        for r in range(nent):
                            mtiles[r] = sb.tile([P, BL], F16, tag="m", bufs=12,
                                                name=f"m{ot}_{r}")
                            derf_m(r, c0)
                            mm_d(r, c0)
                        for r in range(nent):
                            derf_m(r, c1)
                            mm_d(r, c1)
                            if r >= lag:
                                mm_n(r - lag, c0)
                                mm_n(r - lag, c1)
                        for r in range(nent, nfull):
                            mtiles[r] = sb.tile([P, BL], F16, tag="m", bufs=12,
                                                name=f"m{ot}_{r}")
                            derf_m(r, slice(0, BL))
                            mm_d(r, c0)
                            mm_d(r, c1)
                            if r >= lag:
                                mm_n(r - lag, c0)
                                mm_n(r - lag, c1)
                        if lag:
                            for r in range(nfull - lag, nfull):
                                mm_n(r, c0)
                                mm_n(r, c1)
                        if lastot:
                            # trailing rules: all c1 derfs as a block (c1's D/N
                            # close early; its tail chain overlaps the c0
                            # block), then the c0 derfs
                            for r in range(nfull, N_RULES):
                                mtiles[r] = sb.tile([P, BL], F16, tag="m", bufs=12,
                                                    name=f"m{ot}_{r}")
                                derf_m(r, c1)
                                mm_d(r, c1)
                                mm_n(r, c1)
                            for r in range(nfull, N_RULES):
                                derf_m(r, c0)
                                mm_d(r, c0)
                                mm_n(r, c0)

                        orow = out_d[ot * P:(ot + 1) * P, :]
                        if not lastot:
                            # fully overlapped by the next o-tile's derf phase:
                            # everything on DVE, per chunk
                            for ci, cc in enumerate((c0, c1)):
                                rD = sb.tile([P, FD], F32, tag=f"rD{ci}", name=f"rD{ot}c{ci}")
                                w = sb.tile([P, FD], F32, tag=f"w{ci}", name=f"w{ot}c{ci}")
                                o = sb.tile([P, FD], BF16, tag=f"o{ci}", name=f"o{ot}c{ci}")
                                nc.vector.reciprocal_approx_fast(rD[:], Dt[cc][:, :])
                                nc.vector.tensor_tensor(w[:], Nt[cc][:, :], rD[:], Op.mult)
                                for _ in range(5):
                                    nc.vector.tensor_tensor(w[:], w[:], w[:], Op.mult)
                                nc.vector.tensor_tensor(o[:], w[:], P_rep[:, cc], Op.mult)
                                nc.sync.dma_start(out=orow[:, cc], in_=o[:])
                        else:
                            # serial tail, all per-chunk tiles: DVE runs the c1
                            # chain under the c0 derf block; c0's recip/w are
                            # interleaved so ACT's Square chain starts promptly
                            QD = FD // 2
                            q0 = slice(0, QD)
                            q1 = slice(QD, FD)
                            rDa = sb.tile([P, FD], F32, tag="rDa", name="rDc1")
                            wa = sb.tile([P, FD], F32, tag="wa", name="wc1")
                            oa = sb.tile([P, FD], BF16, tag="oa", name="oc1")
                            rq0 = sb.tile([P, QD], F32, tag="rq0", name="rDq0")
                            rq1 = sb.tile([P, QD], F32, tag="rq1", name="rDq1")
                            wq0 = sb.tile([P, QD], F32, tag="wq0", name="wq0")
                            wq1 = sb.tile([P, QD], F32, tag="wq1", name="wq1")
                            oq0 = sb.tile([P, QD], BF16, tag="oq0", name="oq0")
                            oq1 = sb.tile([P, QD], BF16, tag="oq1", name="oq1")
                            # c1 chain: recip/w + 3 squares on DVE under the
                            # trailing c0 derf block; its last 2 squares on ACT
                            nc.vector.reciprocal_approx_fast(rDa[:], Dt[c1][:, :])
                            nc.vector.tensor_tensor(wa[:], Nt[c1][:, :], rDa[:], Op.mult)
                            for _ in range(3):
                                nc.vector.tensor_tensor(wa[:], wa[:], wa[:], Op.mult)
                            # c0 recip/w per quarter (separate tiles: no false
                            # deps between the DVE-q0 and ACT-q1 square chains)
                            nc.vector.reciprocal_approx_fast(rq0[:], Dt[c0][:, q0])
                            nc.vector.tensor_tensor(wq0[:], Nt[c0][:, q0], rq0[:], Op.mult)
                            nc.vector.reciprocal_approx_fast(rq1[:], Dt[c0][:, q1])
                            nc.vector.tensor_tensor(wq1[:], Nt[c0][:, q1], rq1[:], Op.mult)
                            for _ in range(2):
                                nc.scalar.activation(wa[:], wa[:], SQ)
                            for _ in range(5):
                                nc.scalar.activation(wq1[:], wq1[:], SQ)
                            nc.vector.tensor_tensor(oa[:], wa[:], P_rep[:, c1], Op.mult)
                            nc.sync.dma_start(out=orow[:, c1], in_=oa[:])
                            for _ in range(5):
                                nc.vector.tensor_tensor(wq0[:], wq0[:], wq0[:], Op.mult)
                            nc.vector.tensor_tensor(oq0[:], wq0[:], P_rep[:, 0:QD], Op.mult)
                            nc.sync.dma_start(out=orow[:, 0:QD], in_=oq0[:])
                            nc.vector.tensor_tensor(oq1[:], wq1[:], P_rep[:, QD:FD], Op.mult)
                            nc.sync.dma_start(out=orow[:, QD:FD], in_=oq1[:])

    nc.finalize()
    _nc_cache = nc
    return nc


def _prepare_in_maps(x, W1, b1, W2, b2, W3, b3, centers, widths, params):
    x = np.ascontiguousarray(x, dtype=np.float32)
    W1 = np.asarray(W1, np.float32); b1 = np.asarray(b1, np.float32)
    W2 = np.asarray(W2, np.float32); b2 = np.asarray(b2, np.float32)
    W3 = np.asarray(W3, np.float32); b3 = np.asarray(b3, np.float32)
    centers = np.asarray(centers, np.float32)
    widths = np.asarray(widths, np.float32)
    params = np.asarray(params, np.float32)

    def pack_w(W, nj_out):
        blocks = []
        for k in range(W.shape[0] // P):
            for j in range(nj_out):
                blocks.append(W[k * P:(k + 1) * P, j * P:(j + 1) * P])
        return np.concatenate(blocks, axis=1)

    w2 = np.ascontiguousarray(pack_w(W2, NJ).astype(np.float16))
    w3 = np.ascontiguousarray(pack_w(W3, NOT).astype(np.float16))

    b1t = b1.reshape(NJ, P).T
    b2t = b2.reshape(NJ, P).T
    inv = (1.0 / widths).astype(np.float32)
    biasf = ((b3[:, None] - centers) * inv).astype(np.float32)
    scl = inv.reshape(NOT, P, N_RULES).transpose(1, 0, 2).reshape(P, NOT * N_RULES)
    bia = biasf.reshape(NOT, P, N_RULES).transpose(1, 0, 2).reshape(P, NOT * N_RULES)
    cs = np.ascontiguousarray(np.concatenate([b1t, b2t, scl, bia], axis=1))

    ph = params.astype(np.float16)
    dgs = np.zeros((NOT, P, N_RULES * P), np.float16)
    for ot in range(NOT):
        for r in range(N_RULES):
            dgs[ot][:, r * P:(r + 1) * P] = np.diag(ph[ot * P:(ot + 1) * P, r])
    ey = np.ascontiguousarray(np.eye(P, dtype=np.float16))
    dg0 = np.ascontiguousarray(dgs[0])
    dg1 = np.ascontiguousarray(dgs[1])

    # L1 bf16 hi/lo stacking: y1 = W1h.T@xh + W1l.T@xh + W1h.T@xl
    W1h = W1.astype(ml_bf16)
    W1l = (W1 - W1h.astype(np.float32)).astype(ml_bf16)
    w1s = np.ascontiguousarray(np.concatenate([W1h, W1l, W1h], axis=0))  # [96, H]

    in_maps = []
    for i in range(N_CORES):
        xs = x[i * BL:(i + 1) * BL]                              # [BL, 32]
        xT = np.ascontiguousarray(xs.T)                          # [32, BL]
        xh = xT.astype(ml_bf16)
        xl = (xT - xh.astype(np.float32)).astype(ml_bf16)
        xw = np.ascontiguousarray(np.concatenate([xh, xh, xl], axis=0))  # [96, BL]
        xbp = np.ascontiguousarray(
            xs.reshape(BL // P, P, IN_DIM).transpose(1, 0, 2).reshape(P, -1))
        in_maps.append(dict(w1s=w1s, xw=xw, cs=cs, w2=w2, w3=w3, ey=ey,
                            dg0=dg0, xbp=xbp, dg1=dg1))
    return in_maps


def run(trace=False, **inputs):
    nc = _build()
    in_maps = _prepare_in_maps(**inputs)
    res = run_bass_kernel_spmd(nc, in_maps, core_ids=list(range(N_CORES)), trace=trace)
    outs = [np.asarray(res.results[i]["out"]).astype(np.float32).T for i in range(N_CORES)]
    full = np.ascontiguousarray(np.concatenate(outs, axis=0), dtype=np.float32)
    return full, res


def kernel(**inputs) -> np.ndarray:
    full, _ = run(trace=False, **inputs)
    return full


# revision 28
# speedup vs baseline: 1.0508x; 1.0508x over previous
"""ANFIS kernel for 8 TRN2 NeuronCores — pure batch data-parallel.

Math: out[b,o] = prod_f(x[b,f]) * w[b,o]^32 where
  w = sum_r(p_r * m_r) / sum_r(m_r),  m_r = exp(-((y-c_r)/s_r)^2),
  y = MLP(x).  exp(-z^2) is computed on the ScalarEngine as
  Derivative_Erf(scale*y + bias) (= 2/sqrt(pi) * exp(-z^2); the constant
  cancels in the normalization).  D = sum m and N = sum p*m are reduced
  over rules with fp16 TensorE matmuls (identity / diag(p) stationary),
  accumulating in f32 PSUM.
"""
import sys

if "/opt/trn_rl_repo" not in sys.path:
    sys.path.insert(0, "/opt/trn_rl_repo")

import numpy as np
import ml_dtypes
ml_bf16 = ml_dtypes.bfloat16

import concourse.bacc as bacc
import concourse.mybir as mybir
from concourse.bass_utils import run_bass_kernel_spmd
from concourse.tile import TileContext
from concourse.mybir import AluOpType as Op

B, IN_DIM, OUT_DIM, N_RULES, H = 8192, 32, 256, 16, 256
N_CORES = 8
BL = B // N_CORES          # 1024 batch rows per core
P = 128                    # partitions
NOT = OUT_DIM // P         # 2 o-tiles
NJ = H // P                # 2 hidden j-tiles
FD = 512                   # matmul free-dim chunk (one PSUM bank)
F32 = mybir.dt.float32
F16 = mybir.dt.float16

# packed f32 constant columns: b1t | b2t | scl | bia | eyef | xbp
C_B1 = 0
C_B2 = C_B1 + NJ
C_SCL = C_B2 + NJ
C_BIA = C_SCL + NOT * N_RULES
C_EYE = C_BIA + NOT * N_RULES
C_XBP = C_EYE + P
C_END = C_XBP + (BL // P) * IN_DIM

_nc_cache = None


def _build():
    global _nc_cache
    if _nc_cache is not None:
        return _nc_cache
    nc = bacc.Bacc(None, target_bir_lowering=False, debug=False, num_devices=N_CORES)

    xw_d = nc.declare_dram_parameter("xw", [3 * IN_DIM, BL], mybir.dt.bfloat16, isOutput=False)
    w1s_d = nc.declare_dram_parameter("w1s", [3 * IN_DIM, H], mybir.dt.bfloat16, isOutput=False)
    cst_d = nc.declare_dram_parameter("cst", [P, C_END], F32, isOutput=False)
    w23_d = nc.declare_dram_parameter("w23", [P, (NJ * NJ + NJ * NOT) * P], F16, isOutput=False)
    f16c_d = nc.declare_dram_parameter("f16c", [P, P + NOT * N_RULES * P], F16, isOutput=False)
    out_d = nc.declare_dram_parameter("out", [OUT_DIM, BL], F32, isOutput=True)

    DERF = mybir.ActivationFunctionType.Derivative_Erf
    SQ = mybir.ActivationFunctionType.Square
    RELU = mybir.ActivationFunctionType.Relu
    NCH = BL // FD  # chunks

    with TileContext(nc) as tc:
        with tc.sbuf_pool(name="sb", bufs=1) as sb:
            # ---- warm the PE clock gate on garbage data during the DMA wait ----
            junk16 = sb.tile([P, P + 256], F16)
            nc.vector.memset(junk16[:], 0.0)
            # tiny dummy derf as the FIRST ACT op: forces the erf_derivative
            # table set (which also covers Relu/Square) to load during the
            # preamble instead of gating the first real derf
            nc.scalar.activation(junk16[:, 0:8], junk16[:, 0:8], DERF)
            with tc.psum_pool(name="ps_warm", bufs=1) as ps_warm:
                wt = ps_warm.tile([P, 256], F32, tag="warm")
                for _ in range(14):
                    nc.tensor.matmul(wt[:], junk16[:, :P], junk16[:, P:], start=True, stop=True)

            # ---- loads: xw/w1s first so L1 starts ASAP ----
            xw = sb.tile([3 * IN_DIM, BL], mybir.dt.bfloat16)
            nc.sync.dma_start(out=xw[:], in_=xw_d[:])
            w1s = sb.tile([3 * IN_DIM, H], mybir.dt.bfloat16)
            nc.sync.dma_start(out=w1s[:], in_=w1s_d[:])
            cst = sb.tile([P, C_END], F32)
            nc.sync.dma_start(out=cst[:], in_=cst_d[:])
            w23 = sb.tile([P, (NJ * NJ + NJ * NOT) * P], F16)
            nc.sync.dma_start(out=w23[:], in_=w23_d[:])
            f16c = sb.tile([P, P + NOT * N_RULES * P], F16)
            nc.sync.dma_start(out=f16c[:], in_=f16c_d[:])

            b1t = cst[:, C_B1:C_B1 + NJ]
            b2t = cst[:, C_B2:C_B2 + NJ]
            scl = cst[:, C_SCL:C_SCL + NOT * N_RULES]
            bia = cst[:, C_BIA:C_BIA + NOT * N_RULES]
            xbp = cst[:, C_XBP:C_END]
            eye16 = f16c[:, :P]
            dgs = f16c[:, P:]

            def W2blk(k, j):
                return w23[:, (k * NJ + j) * P:(k * NJ + j + 1) * P]

            def W3blk(k, j):
                off = NJ * NJ * P
                return w23[:, off + (k * NOT + j) * P:off + (k * NOT + j + 1) * P]

            # ---- P[b] = prod_f x[b,f], replicated across partitions ----
            # P_all[p, t] = P(b = 128*t + p); DMA-permute into one row, then
            # gpsimd partition-broadcast to [128, BL]. No PSUM, no TensorE.
            P_rep = sb.tile([P, BL], F32)
            P_all = sb.tile([P, BL // P], F32)
            nc.vector.tensor_reduce(
                P_all[:],
                xbp.rearrange("p (t f) -> p t f", f=IN_DIM),
                mybir.AxisListType.X, Op.mult,
            )
            P_row = sb.tile([1, BL], F32)
            for t in range(BL // P):
                nc.sync.dma_start(out=P_row[0:1, t * P:(t + 1) * P], in_=P_all[:, t:t + 1])
            nc.gpsimd.partition_broadcast(P_rep[:], P_row[0:1, :])

            def relu_bias(dst, src_psum, bias_col, j):
                # one full-tile op per engine so the two j-tiles run in parallel
                if j % 2 == 0:
                    nc.vector.tensor_scalar(dst[:], src_psum[:], bias_col, 0.0,
                                            Op.add, Op.max)
                else:
                    nc.scalar.activation(dst[:], src_psum[:], RELU,
                                         bias=bias_col, scale=1.0)

            def cs_p(cs):
                return (slice(None), cs)

            hT = []
            h2T = []
            with tc.psum_pool(name="ps_y", bufs=2) as ps_y:
                yT = []
                with tc.psum_pool(name="ps_mlp", bufs=2) as ps_mlp:
                    for j in range(NJ):
                        l1 = ps_mlp.tile([P, BL], F32, tag="mlp")
                        for c in range(NCH):
                            nc.tensor.matmul(
                                l1[:, c * FD:(c + 1) * FD],
                                w1s[:, j * P:(j + 1) * P],
                                xw[:, c * FD:(c + 1) * FD],
                                start=True, stop=True,
                            )
                        h = sb.tile([P, BL], F16, name=f"hT{j}")
                        relu_bias(h[:], l1[:], b1t[:, j:j + 1], j)
                        hT.append(h)
                    for j in range(NJ):
                        l2 = ps_mlp.tile([P, BL], F32, tag="mlp")
                        for c in range(NCH):
                            for k in range(NJ):
                                nc.tensor.matmul(
                                    l2[:, c * FD:(c + 1) * FD],
                                    W2blk(k, j),
                                    hT[k][:, c * FD:(c + 1) * FD],
                                    start=(k == 0), stop=(k == NJ - 1),
                                )
                        h = sb.tile([P, BL], F16, name=f"h2T{j}")
                        relu_bias(h[:], l2[:], b2t[:, j:j + 1], j)
                        h2T.append(h)
                    for j in range(NOT):
                        l3 = ps_y.tile([P, BL], F32, tag="yt")
                        for c in range(NCH):
                            for k in range(NJ):
                                nc.tensor.matmul(
                                    l3[:, c * FD:(c + 1) * FD],
                                    W3blk(k, j),
                                    h2T[k][:, c * FD:(c + 1) * FD],
                                    start=(k == 0), stop=(k == NJ - 1),
                                )
                        yT.append(l3)

                # ---- memberships + D/N + w per o-tile ----
                with tc.psum_pool(name="ps_dn", bufs=1) as ps_dn:
                    for ot in range(NOT):
                        D = ps_dn.tile([P, BL], F32, tag="D", name=f"D{ot}")
                        N = ps_dn.tile([P, BL], F32, tag="N", name=f"N{ot}")
                        for r in range(N_RULES):
                            idx = ot * N_RULES + r
                            m = sb.tile([P, BL], F16, tag="m", bufs=6, name=f"m{idx}")
                            if ot == NOT - 1 and r == N_RULES - 1:
                                for c in range(NCH):
                                    cs = slice(c * FD, (c + 1) * FD)
                                    nc.scalar.activation(
                                        m[:, cs], yT[ot][:, cs], DERF,
                                        bias=bia[:, idx:idx + 1], scale=scl[:, idx:idx + 1],
                                    )
                            else:
                                nc.scalar.activation(
                                    m[:], yT[ot][:], DERF,
                                    bias=bia[:, idx:idx + 1], scale=scl[:, idx:idx + 1],
                                )
                            for c in range(NCH):
                                cs = slice(c * FD, (c + 1) * FD)
                                nc.tensor.matmul(D[:, cs], eye16, m[:, cs],
                                                 start=(r == 0), stop=(r == N_RULES - 1))
                                nc.tensor.matmul(N[:, cs], dgs[:, idx * P:(idx + 1) * P], m[:, cs],
                                                 start=(r == 0), stop=(r == N_RULES - 1))
                        rD = sb.tile([P, BL], F32, tag="rD", bufs=2, name=f"rD{ot}")
                        nc.vector.reciprocal_approx_fast(rD[:], D[:])
                        w = sb.tile([P, BL], F32, tag="w", bufs=2, name=f"w{ot}")
                        nc.vector.tensor_tensor(w[:], N[:], rD[:], Op.mult)
                        o = sb.tile([P, BL], F32, tag="osb", bufs=2, name=f"osb{ot}")
                        if ot < NOT - 1:
                            for _ in range(5):
                                nc.vector.tensor_tensor(w[:], w[:], w[:], Op.mult)
                            nc.vector.tensor_tensor(o[:], w[:], P_rep[:], Op.mult)
                            nc.sync.dma_start(out=out_d[ot * P:(ot + 1) * P, :], in_=o[:])
                        else:
                            # last o-tile = serial tail: pipeline halves on DVE + ACT
                            h0 = slice(0, BL // 2)
                            h1 = slice(BL // 2, BL)
                            for _ in range(5):
                                nc.vector.tensor_tensor(w[:, h0], w[:, h0], w[:, h0], Op.mult)
                                nc.scalar.activation(w[:, h1], w[:, h1], SQ)
                            nc.vector.tensor_tensor(o[:, h0], w[:, h0], P_rep[:, h0], Op.mult)
                            nc.sync.dma_start(out=out_d[ot * P:(ot + 1) * P, :BL // 2], in_=o[:, h0])
                            nc.vector.tensor_tensor(o[:, h1], w[:, h1], P_rep[:, h1], Op.mult)
                            nc.sync.dma_start(out=out_d[ot * P:(ot + 1) * P, BL // 2:], in_=o[:, h1])

    nc.finalize()
    _nc_cache = nc
    return nc


def _prepare_in_maps(x, W1, b1, W2, b2, W3, b3, centers, widths, params):
    x = np.ascontiguousarray(x, dtype=np.float32)
    W1 = np.asarray(W1, np.float32); b1 = np.asarray(b1, np.float32)
    W2 = np.asarray(W2, np.float32); b2 = np.asarray(b2, np.float32)
    W3 = np.asarray(W3, np.float32); b3 = np.asarray(b3, np.float32)
    centers = np.asarray(centers, np.float32)
    widths = np.asarray(widths, np.float32)
    params = np.asarray(params, np.float32)

    def pack_w(W, nj_out):
        blocks = []
        for k in range(W.shape[0] // P):
            for j in range(nj_out):
                blocks.append(W[k * P:(k + 1) * P, j * P:(j + 1) * P])
        return np.concatenate(blocks, axis=1)

    w23 = np.ascontiguousarray(
        np.concatenate([pack_w(W2, NJ), pack_w(W3, NOT)], axis=1).astype(np.float16))

    b1t = b1.reshape(NJ, P).T
    b2t = b2.reshape(NJ, P).T
    inv = (1.0 / widths).astype(np.float32)
    biasf = ((b3[:, None] - centers) * inv).astype(np.float32)
    scl = inv.reshape(NOT, P, N_RULES).transpose(1, 0, 2).reshape(P, NOT * N_RULES)
    bia = biasf.reshape(NOT, P, N_RULES).transpose(1, 0, 2).reshape(P, NOT * N_RULES)
    eyef = np.eye(P, dtype=np.float32)

    ph = params.astype(np.float16)
    dgs = np.zeros((P, NOT * N_RULES * P), np.float16)
    for ot in range(NOT):
        for r in range(N_RULES):
            idx = ot * N_RULES + r
            dgs[:, idx * P:(idx + 1) * P] = np.diag(ph[ot * P:(ot + 1) * P, r])
    f16c = np.ascontiguousarray(np.concatenate([np.eye(P, dtype=np.float16), dgs], axis=1))

    # L1 bf16 hi/lo stacking: y1 = W1h.T@xh + W1l.T@xh + W1h.T@xl
    W1h = W1.astype(ml_bf16)
    W1l = (W1 - W1h.astype(np.float32)).astype(ml_bf16)
    w1s = np.ascontiguousarray(np.concatenate([W1h, W1l, W1h], axis=0))  # [96, H]

    in_maps = []
    for i in range(N_CORES):
        xs = x[i * BL:(i + 1) * BL]                              # [BL, 32]
        xT = np.ascontiguousarray(xs.T)                          # [32, BL]
        xh = xT.astype(ml_bf16)
        xl = (xT - xh.astype(np.float32)).astype(ml_bf16)
        xw = np.ascontiguousarray(np.concatenate([xh, xh, xl], axis=0))  # [96, BL]
        xbp = xs.reshape(BL // P, P, IN_DIM).transpose(1, 0, 2).reshape(P, -1)
        cst = np.ascontiguousarray(
            np.concatenate([b1t, b2t, scl, bia, eyef, xbp], axis=1))
        in_maps.append(dict(xw=xw, w1s=w1s, cst=cst, w23=w23, f16c=f16c))
    return in_maps


def run(trace=False, **inputs):
    nc = _build()
    in_maps = _prepare_in_maps(**inputs)
    res = run_bass_kernel_spmd(nc, in_maps, core_ids=list(range(N_CORES)), trace=trace)
    outs = [res.results[i]["out"].T for i in range(N_CORES)]     # each [BL, O]
    full = np.ascontiguousarray(np.concatenate(outs, axis=0), dtype=np.float32)
    return full, res


def kernel(**inputs) -> np.ndarray:
    full, _ = run(trace=False, **inputs)
    return full

